# revision 2
# baseline (speedup 1.0000x reference)
"""BertAttention (relative_key_query) Trainium2 Bass kernel — v2.

Data-parallel over batch (core i = batch i). Per-core changes vs v1:
  - bf16 matmul operands everywhere (f32 PSUM accumulate); fp8 staging.
  - contract-64 matmuls issued as row-group pairs (partitions 0-63 /
    64-127) so the PE runs them concurrently via tile_position.
  - merged multi-block DMAs (staging writes, skew reads, weight loads)
    to cut DGE descriptor-generation cost (~1us per dma_start).
  - wide single-op PSUM evacuations ([128,2048] across 4 banks),
    split between DVE and Act.
  - scores pipeline: skew DMAs -> quarter 0 -> stage next pair (b0-3)
    -> quarter 1 -> stage next pair (b4-7).
"""

import sys
from contextlib import ExitStack

sys.path.insert(0, "/opt/trn_rl_repo")

import numpy as np

import concourse.bass as bass
import concourse.tile as tile
from concourse import bacc, mybir

F32 = mybir.dt.float32
BF16 = mybir.dt.bfloat16
F8 = mybir.dt.float8e4
AF = mybir.ActivationFunctionType

S = 1024
H = 16
D = 64
E = H * D
N_CORES = 8
EPS = 1e-12
WIN = S + 128          # staged window width per 128-row block
NBLK = S // 128        # 8
NCH = E // 128         # 8
NPAIR = H // 2         # 8 head pairs (2 heads per 128-chunk)
LQ = 512               # l-quarter width
NQ = S // LQ           # 2
TABW = 2 * S           # table width (2S-1 real cols + zero pad)


def rep_ap(t, parts, n):
    return bass.AP(t, 0, [[0, parts], [1, n]])


def build_program(nrep=1):
    s, d = S, D
    nc = bacc.Bacc(None)

    # ---- external I/O ----
    # hs/Wq/Wk/Wv in fp8 for DoubleRow projections; W pre-scaled x64 on
    # the host (fp8 subnormal escape), undone by scale=1/64 at bias-add.
    hsT_d = nc.dram_tensor("hsT", [E, s], F8, kind="ExternalInput")
    hsr_d = nc.dram_tensor("hs_res", [s, E], F32, kind="ExternalInput")
    wq_d = nc.dram_tensor("wq8", [E, E], F8, kind="ExternalInput")
    wk_d = nc.dram_tensor("wk", [E, E], F8, kind="ExternalInput")
    wv_d = nc.dram_tensor("wv", [E, E], F8, kind="ExternalInput")
    wo_d = nc.dram_tensor("wo", [E, E], BF16, kind="ExternalInput")
    et_d = nc.dram_tensor("et", [d, TABW], BF16, kind="ExternalInput")
    ehat_d = nc.dram_tensor("ehatT", [d, TABW], BF16, kind="ExternalInput")
    id16_d = nc.dram_tensor("id16", [128, 128], F8, kind="ExternalInput")
    bq_d = nc.dram_tensor("bq8", [NCH, 128], F32, kind="ExternalInput")
    bk_d = nc.dram_tensor("bk", [NCH, 128], F32, kind="ExternalInput")
    bv_d = nc.dram_tensor("bv", [1, E], F32, kind="ExternalInput")
    gam_d = nc.dram_tensor("gamma", [1, E], F32, kind="ExternalInput")
    bet_d = nc.dram_tensor("beta", [1, E], F32, kind="ExternalInput")
    out_d = nc.dram_tensor("out", [s, E], F32, kind="ExternalOutput")

    # ---- internal DRAM staging per head pair: [2 hb, 128, 8*WIN] fp8 ----
    wstg = [nc.dram_tensor(f"wstg{p}", [2, 128, NBLK * WIN], F8)
            for p in range(NPAIR)]
    ystg = [nc.dram_tensor(f"ystg{p}", [2, 128, NBLK * WIN], F8)
            for p in range(NPAIR)]
    dend = [[nc.dram_tensor(f"dend_{p}_{v}", [1, 2 * LQ], F32)
             for v in range(NQ)] for p in range(NPAIR)]

    with tile.TileContext(nc) as tc, ExitStack() as stk:
        rep_cm = tc.For_i(0, nrep, 1) if nrep > 1 else None
        if rep_cm is not None:
            rep_cm.__enter__()
        consts = stk.enter_context(tc.tile_pool(name="consts", bufs=1))
        v_pool = stk.enter_context(tc.tile_pool(name="vp", bufs=1))
        ctx_pool = stk.enter_context(tc.tile_pool(name="ctxp", bufs=1))
        qk_stk = ExitStack()
        qk_pool = qk_stk.enter_context(tc.tile_pool(name="qkp", bufs=1))

        # ---------- constants ----------
        # zero-padded stacked tables: tbz[:, hb, :] has the table on
        # partition rows hb*64..hb*64+63 and zeros elsewhere, so a
        # contract-128 matmul against a 2-head-stacked q/k block computes
        # exactly one head's window product.
        ehat = consts.tile([128, 2, TABW], BF16, tag="ehat", name="ehat")
        et = consts.tile([128, 2, TABW], BF16, tag="et", name="et")
        nc.vector.memset(ehat, 0.0)
        nc.vector.memset(et, 0.0)
        for half in range(2):
            p0 = half * 64
            nc.sync.dma_start(ehat[p0:p0 + 64, half, :], ehat_d[:, :])
            nc.sync.dma_start(et[p0:p0 + 64, half, :], et_d[:, :])
        id16 = consts.tile([128, 128], F8, tag="id16", name="id16")
        nc.sync.dma_start(id16, id16_d[:, :])
        bq_sb = consts.tile([128, NCH], F32, tag="bq", name="bq")
        bk_sb = consts.tile([128, NCH], F32, tag="bk", name="bk")
        for m in range(NCH):
            nc.gpsimd.dma_start(bq_sb[:, m:m + 1], bq_d[m, :])
            nc.gpsimd.dma_start(bk_sb[:, m:m + 1], bk_d[m, :])
        eps_sb = consts.tile([128, 1], F32, tag="eps", name="eps")
        nc.vector.memset(eps_sb, EPS)

        # ---------- persistent activations ----------
        qT = [qk_pool.tile([128, s], BF16, tag=f"qT{c}", name=f"qT{c}")
              for c in range(NCH)]
        kT = [qk_pool.tile([128, s], BF16, tag=f"kT{c}", name=f"kT{c}")
              for c in range(NCH)]
        # v with a ones column per head: [128 r, 16h * 65]
        vaug = [v_pool.tile([128, H * 65], BF16, tag=f"v{r}", name=f"v{r}")
                for r in range(NBLK)]

        # ---------- stage A: load hs^T + weights, project q/k/v ----------
        with tc.tile_pool(name="ab", bufs=1) as ab, \
                tc.tile_pool(name="ps512", bufs=3, space="PSUM") as ps512:
            bv_sb = ab.tile([128, E], F32, tag="bv", name="bv")
            nc.gpsimd.dma_start(bv_sb, rep_ap(bv_d, 128, E))
            hsTt = ab.tile([128, NCH, s], F8, tag="hsTt", name="hsTt")
            for c in range(NCH):
                nc.sync.dma_start(hsTt[:, c, :],
                                  hsT_d[c * 128:(c + 1) * 128, :])
            DR = mybir.MatmulPerfMode.DoubleRow

            def project_T(w_dram, bias_sb, dest):
                for m in range(NCH):
                    wcol = ab.tile([128, NCH, 128], F8, tag="wcol",
                                   name="wcol", bufs=2)
                    nc.sync.dma_start(
                        wcol,
                        bass.AP(w_dram, m * 128,
                                [[E, 128], [128 * E, NCH], [1, 128]]))
                    for n in range(NQ):
                        ps = ps512.tile([128, 512], F32, tag="ps512",
                                        name="ps512")
                        for j in range(NCH // 2):
                            nc.tensor.matmul(
                                ps,
                                lhsT=wcol[:, 2 * j:2 * j + 2, :],
                                rhs=hsTt[:, 2 * j:2 * j + 2,
                                         n * 512:(n + 1) * 512],
                                start=(j == 0), stop=(j == NCH // 2 - 1),
                                perf_mode=DR)
                        nc.scalar.activation(
                            dest[m][:, n * 512:(n + 1) * 512],
                            ps, AF.Identity, bias=bias_sb[:, m:m + 1],
                            scale=1.0 / 64.0)

            project_T(wq_d, bq_sb, qT)
            project_T(wk_d, bk_sb, kT)

            for n in range(2):
                wvb = ab.tile([128, NCH, 512], F8, tag="wvb", name="wvb",
                              bufs=2)
                nc.sync.dma_start(
                    wvb,
                    bass.AP(wv_d, n * 512,
                            [[E, 128], [128 * E, NCH], [1, 512]]))
                for r in range(NBLK):
                    ps = ps512.tile([128, 512], F32, tag="ps512", name="ps512")
                    for j in range(NCH // 2):
                        nc.tensor.matmul(
                            ps,
                            lhsT=hsTt[:, 2 * j:2 * j + 2,
                                      r * 128:(r + 1) * 128],
                            rhs=wvb[:, 2 * j:2 * j + 2, :],
                            start=(j == 0), stop=(j == NCH // 2 - 1),
                            perf_mode=DR)
                    vv = vaug[r].rearrange("p (h x) -> p h x", x=65)
                    hpn = 8  # heads per 512-slice
                    nc.vector.scalar_tensor_tensor(
                        vv[:, n * hpn:(n + 1) * hpn, 0:64],
                        ps.rearrange("p (h x) -> p h x", x=64),
                        1.0 / 64.0,
                        bv_sb[:, n * 512:(n + 1) * 512]
                        .rearrange("p (h x) -> p h x", x=64),
                        op0=mybir.AluOpType.mult,
                        op1=mybir.AluOpType.add)
            for r in range(NBLK):
                vv = vaug[r].rearrange("p (h x) -> p h x", x=65)
                nc.vector.memset(vv[:, :, 64:65], 1.0)

        # ---------- stage C: per-head-pair attention ----------
        ctxT = [ctx_pool.tile([128, s], BF16, tag=f"cx{c}", name=f"cx{c}")
                for c in range(NCH)]
        cstk = ExitStack()
        stg_ps = cstk.enter_context(
            tc.tile_pool(name="stg_ps", bufs=2, space="PSUM"))
        ps_sc = cstk.enter_context(
            tc.tile_pool(name="ps_sc", bufs=1, space="PSUM"))
        ps_ctx = cstk.enter_context(
            tc.tile_pool(name="ps_ctx", bufs=1, space="PSUM"))
        stg_sb = cstk.enter_context(tc.tile_pool(name="stg_sb", bufs=1))
        skew_p = cstk.enter_context(tc.tile_pool(name="skew", bufs=1))
        prob_p = cstk.enter_context(tc.tile_pool(name="prob", bufs=1))
        den_p = cstk.enter_context(tc.tile_pool(name="den", bufs=2))
        cxb_p = cstk.enter_context(tc.tile_pool(name="cxb", bufs=2))

        nev = [0]  # evac round-robin counter

        def stage_blocks(ch, tbl, stg_d, blocks, kind):
            """Stage W (tbl=ehat) or Y (tbl=et) windows for head pair ch.

            Contract-128 matmuls: stationary = both heads' 64-dims stacked,
            moving = zero-padded table slice for one head."""
            src = qT[ch] if kind == "w" else kT[ch]
            half = 0 if blocks[0] == 0 else 1
            stg = stg_sb.tile([128, 2, 4 * WIN], F8, tag=f"stg_{kind}",
                              name=f"stg_{kind}", bufs=1)
            for b in blocks:
                c_lo = (s - 128) - b * 128
                bh = b - half * 4
                for n0, nw in ((0, 512), (512, 512), (1024, 128)):
                    pss = [stg_ps.tile([128, 512], F32, tag="stgps",
                                       name="stgps") for _ in range(2)]
                    for hb in range(2):
                        nc.tensor.matmul(
                            pss[hb][:, :nw],
                            lhsT=src[:, b * 128:(b + 1) * 128],
                            rhs=tbl[:, hb, c_lo + n0:c_lo + n0 + nw],
                            start=True, stop=True)
                    for hb in range(2):
                        dst = stg[:, hb, bh * WIN + n0:bh * WIN + n0 + nw]
                        if nev[0] % 12 < 5:
                            nc.scalar.activation(dst, pss[hb][:, :nw],
                                                 AF.Copy)
                        else:
                            nc.vector.tensor_copy(dst, pss[hb][:, :nw])
                        nev[0] += 1
            for hb in range(2):
                dst = bass.AP(stg_d,
                              hb * (128 * NBLK * WIN) + half * 4 * WIN,
                              [[NBLK * WIN, 128], [1, 4 * WIN]])
                nc.sync.dma_start(dst, stg[:, hb, :])

        def skew_reads(ch):
            """Merged diagonal reads: one 3-dim-AP DMA per (term, hb)."""
            rq, rk = [], []
            for hb in range(2):
                t = skew_p.tile([128, NBLK * s], F8, tag=f"rq{hb}",
                                name=f"rq{hb}", bufs=2)
                nc.sync.dma_start(
                    t, bass.AP(wstg[ch], hb * 128 * NBLK * WIN + 127,
                               [[NBLK * WIN - 1, 128], [WIN, NBLK], [1, s]]))
                rq.append(t)
                t = skew_p.tile([128, NBLK * s], F8, tag=f"rk{hb}",
                                name=f"rk{hb}", bufs=2)
                nc.scalar.dma_start(
                    t, bass.AP(ystg[ch], hb * 128 * NBLK * WIN + 127,
                               [[NBLK * WIN - 1, 128], [WIN, NBLK], [1, s]]))
                rk.append(t)
            return rq, rk

        def score_quarter(ch, v_i, rq, rk, qz):
            lsl = slice(v_i * LQ, (v_i + 1) * LQ)
            probs = [[], []]
            for r in range(NBLK):
                pss = [ps_sc.tile([128, LQ], F32, tag=f"sc{hb}_{r % 2}",
                                  name=f"sc{hb}") for hb in range(2)]
                for hb in range(2):
                    nc.tensor.matmul(
                        pss[hb],
                        lhsT=kT[ch][:, r * 128:(r + 1) * 128],
                        rhs=qz[:, hb, lsl],
                        start=True, stop=False)
                for hb in range(2):
                    for j in range(LQ // 128):
                        b = v_i * (LQ // 128) + j
                        nc.tensor.matmul(
                            pss[hb][:, j * 128:(j + 1) * 128],
                            lhsT=rq[hb][:, b * s + r * 128:
                                        b * s + (r + 1) * 128],
                            rhs=id16,
                            start=False, stop=False)
                    nc.tensor.matmul(
                        pss[hb], lhsT=id16,
                        rhs=rk[hb][:, r * s + v_i * LQ:r * s + v_i * LQ + LQ],
                        start=False, stop=True)
                for hb in range(2):
                    pb = prob_p.tile([128, LQ], F8, tag=f"pb{hb}_{r}",
                                     name=f"pb{hb}_{r}")
                    nc.scalar.activation(pb, pss[hb], AF.Exp)
                    probs[hb].append(pb)
            # PV + denominators + ctx (evac-copy pc out of psum so the
            # bank frees immediately; normalize later from sbuf at 2x)
            rden = den_p.tile([128, 2 * LQ], F32, tag="rden", name="rden",
                              bufs=1)
            cus = []
            for hb in range(2):
                h = 2 * ch + hb
                pc = ps_ctx.tile([65, LQ], F32, tag="ctx", name="ctx",
                                 bufs=2)
                for r in range(NBLK):
                    nc.tensor.matmul(
                        pc,
                        lhsT=vaug[r][:, h * 65:(h + 1) * 65],
                        rhs=probs[hb][r],
                        start=(r == 0), stop=(r == NBLK - 1))
                nc.vector.reciprocal(rden[64:65, hb * LQ:(hb + 1) * LQ],
                                     pc[64:65, :])
                cu = den_p.tile([64, LQ], F32, tag=f"cu{hb}",
                                name=f"cu{hb}", bufs=2)
                nc.vector.tensor_copy(cu, pc[0:64, :])
                cus.append(cu)
            nc.gpsimd.dma_start(dend[ch][v_i][:, :], rden[64:65, :])
            rrep = den_p.tile([128, 2 * LQ], F32, tag="rrep", name="rrep",
                              bufs=1)
            nc.gpsimd.dma_start(rrep[0:64, :],
                                rep_ap(dend[ch][v_i], 64, 2 * LQ))
            nc.vector.tensor_mul(ctxT[ch][0:64, lsl], cus[0],
                                 rrep[0:64, 0:LQ])
            cb = cxb_p.tile([128, LQ], BF16, tag="cb", name="cb")
            nc.vector.tensor_mul(cb[0:64, :], cus[1],
                                 rrep[0:64, LQ:2 * LQ])
            nc.sync.dma_start(ctxT[ch][64:128, lsl], cb[0:64, :])

        # zero-padded stacked q (one head live per hb slice); the zero
        # halves are written once and never touched again.
        qzs = [ctx_pool.tile([128, 2, s], BF16, tag=f"qz{i}", name=f"qz{i}")
               for i in range(2)]
        for t in qzs:
            nc.vector.memset(t, 0.0)

        # software pipeline: stage pair 0, then per pair: q0 ->
        # stage next (b0-3) -> q1 -> stage next (b4-7) -> skew reads
        # for next pair (early issue, double-buffered skew tiles)
        stage_blocks(0, ehat, wstg[0], [0, 1, 2, 3], "w")
        stage_blocks(0, et, ystg[0], [0, 1, 2, 3], "y")
        stage_blocks(0, ehat, wstg[0], [4, 5, 6, 7], "w")
        stage_blocks(0, et, ystg[0], [4, 5, 6, 7], "y")
        rq, rk = skew_reads(0)
        for ch in range(NPAIR):
            qz = qzs[ch % 2]
            nc.vector.tensor_copy(qz[0:64, 0, :], qT[ch][0:64, :])
            nc.vector.tensor_copy(qz[64:128, 1, :], qT[ch][64:128, :])
            score_quarter(ch, 0, rq, rk, qz)
            if ch + 1 < NPAIR:
                stage_blocks(ch + 1, ehat, wstg[ch + 1], [0, 1, 2, 3], "w")
                stage_blocks(ch + 1, et, ystg[ch + 1], [0, 1, 2, 3], "y")
            score_quarter(ch, 1, rq, rk, qz)
            if ch + 1 < NPAIR:
                stage_blocks(ch + 1, ehat, wstg[ch + 1], [4, 5, 6, 7], "w")
                stage_blocks(ch + 1, et, ystg[ch + 1], [4, 5, 6, 7], "y")
                rq, rk = skew_reads(ch + 1)

        cstk.close()
        qk_stk.close()

        # ---------- stage D: out projection + residual + LayerNorm ----------
        with tc.tile_pool(name="dstage", bufs=1) as dp, \
                tc.tile_pool(name="dtmp", bufs=2) as dtmp, \
                tc.tile_pool(name="dst", bufs=4) as dst, \
                tc.tile_pool(name="psd", bufs=3, space="PSUM") as psd:
            wo_sb = [dp.tile([128, E], BF16, tag=f"wo{c}", name=f"wo{c}")
                     for c in range(NCH)]
            for c in range(NCH):
                nc.sync.dma_start(wo_sb[c], wo_d[c * 128:(c + 1) * 128, :])
            gam_sb = dp.tile([128, E], F32, tag="gam", name="gam")
            bet_sb = dp.tile([128, E], F32, tag="bet", name="bet")
            nc.gpsimd.dma_start(gam_sb, rep_ap(gam_d, 128, E))
            nc.gpsimd.dma_start(bet_sb, rep_ap(bet_d, 128, E))
            for m in range(s // 128):
                osb = dtmp.tile([128, E], F32, tag="osb", name="osb")
                hres = dtmp.tile([128, E], F32, tag="hres", name="hres")
                nc.gpsimd.dma_start(hres, hsr_d[m * 128:(m + 1) * 128, :])
                for n in range(NQ):
                    ps = psd.tile([128, 512], F32, tag="psd", name="psd")
                    for c in range(NCH):
                        nc.tensor.matmul(
                            ps,
                            lhsT=ctxT[c][:, m * 128:(m + 1) * 128],
                            rhs=wo_sb[c][:, n * 512:(n + 1) * 512],
                            start=(c == 0), stop=(c == NCH - 1))
                    nsl = slice(n * 512, (n + 1) * 512)
                    nc.vector.tensor_add(osb[:, nsl], ps, hres[:, nsl])
                stats = dst.tile([128, 2, 6], F32, tag="st", name="st")
                for g in range(2):
                    nc.vector.bn_stats(stats[:, g, :],
                                       osb[:, g * 512:(g + 1) * 512])
                mv = dst.tile([128, 2], F32, tag="mv", name="mv")
                nc.vector.bn_aggr(mv, stats)
                sd = dst.tile([128, 1], F32, tag="sd", name="sd")
                nc.scalar.activation(sd, mv[:, 1:2], AF.Sqrt, bias=eps_sb)
                rsig = dst.tile([128, 1], F32, tag="rs", name="rs")
                nc.vector.reciprocal(rsig, sd)
                tnorm = dtmp.tile([128, E], F32, tag="tn", name="tn")
                nc.vector.tensor_scalar(tnorm, osb, mv[:, 0:1], rsig,
                                        op0=mybir.AluOpType.subtract,
                                        op1=mybir.AluOpType.mult)
                nc.gpsimd.tensor_mul(tnorm, tnorm, gam_sb)
                nc.gpsimd.tensor_add(tnorm, tnorm, bet_sb)
                nc.gpsimd.dma_start(out_d[m * 128:(m + 1) * 128, :], tnorm)

        if rep_cm is not None:
            rep_cm.__exit__(None, None, None)

    nc.finalize()
    return nc


def host_prep(hidden_states, Wq, bq, Wk, bk, Wv, bv, dist_emb, Wo, bo,
              ln_gamma, ln_beta):
    import ml_dtypes
    bf16 = ml_dtypes.bfloat16
    f8 = ml_dtypes.float8_e4m3
    B = hidden_states.shape[0]
    hidden_states = np.asarray(hidden_states, np.float32)
    dist_emb = np.asarray(dist_emb, np.float32)
    scale = np.float32(1.0 / 8.0)
    # fp8 projection weights are host-scaled x64 (undone on device by
    # the post-matmul scale=1/64) to stay out of fp8 subnormals.
    shared = {
        "wq8": (np.asarray(Wq, np.float32) * scale * 64.0).astype(f8),
        "wk": (np.asarray(Wk, np.float32) * 64.0).astype(f8),
        "wv": (np.asarray(Wv, np.float32) * 64.0).astype(f8),
        "wo": np.asarray(Wo, np.float32).astype(bf16),
        "et": np.concatenate([dist_emb.T, np.zeros((D, 1), np.float32)],
                             axis=1).astype(bf16),
        "ehatT": np.concatenate([dist_emb[::-1].T * 8.0,
                                 np.zeros((D, 1), np.float32)],
                                axis=1).astype(bf16),
        "id16": (np.eye(128) * 0.125).astype(f8),
        "bq8": np.ascontiguousarray(
            (np.asarray(bq, np.float32) * scale).reshape(NCH, 128)),
        "bk": np.ascontiguousarray(
            np.asarray(bk, np.float32).reshape(NCH, 128)),
        "bv": np.asarray(bv, np.float32).reshape(1, E),
        "gamma": np.asarray(ln_gamma, np.float32).reshape(1, E),
        "beta": np.asarray(ln_beta, np.float32).reshape(1, E),
    }
    bo = np.asarray(bo, np.float32)
    in_maps = []
    for b in range(B):
        hs = np.ascontiguousarray(hidden_states[b])
        m = dict(shared)
        m["hsT"] = np.ascontiguousarray(hs.T).astype(f8)
        m["hs_res"] = hs + bo[None, :]
        in_maps.append(m)
    return in_maps


_CACHE = {}


def _get_program():
    if "nc" not in _CACHE:
        _CACHE["nc"] = build_program()
    return _CACHE["nc"]


def kernel(**inputs):
    from concourse.bass_utils import run_bass_kernel_spmd
    nc = _get_program()
    in_maps = host_prep(**inputs)
    res = run_bass_kernel_spmd(nc, in_maps, list(range(N_CORES)))
    out = np.stack([res.results[i]["out"] for i in range(N_CORES)], axis=0)
    return out.astype(np.float32)
